# revision 27
# baseline (speedup 1.0000x reference)
"""Trainium2 Bass kernel for nn_Attention_69483980914985.

Model: bidirectional LSTM (L=2048 steps, H=1024) over a sequence whose input
is a constant vector (mean of target-range embeddings) at every step except
one per direction, followed by softmax attention pooling and a 3-way linear
head.

Because the LSTM input is constant almost everywhere and the gate dynamics
are contractive (rate ~0.89/step), the hidden state converges to a fixed
point in ~56 steps.  The kernel runs T real steps per direction on device,
verifies convergence from device outputs, and folds the (identical) tail
rows into the attention softmax exactly via a multiplicity weight on the
last computed row.

Execution: fully replicated across the 8 cores (a per-step AllGather costs
~26us — far more than the whole replicated 4096-gate step).  Per step the
PE streams all 256 [128x128] fp16 weight tiles (~31-34ns each, instruction-
delivery limited); the activation/cell-update tail is computed in two
half-groups (B = h-blocks 4..7, A = 0..3) whose PSUM accumulation chains
live in four separate single-bank tiles, split by kk range so each chain
stays contiguous (4 MMs) and the next step's first pass depends only on
tail B's h columns:

  step t:  [B:kk4-7][A:kk4-7][B:kk0-3][A:kk0-3]  [B':kk4-7]...
  tails:                     [tail B..........]  [tail A....]

w_ih contributions are step-constant and folded on host; the attention tail
(rows >= T-1, all identical after convergence) enters the softmax with
multiplicity (L - T + 1).
"""

import sys

sys.path.insert(0, "/opt/trn_rl_repo")

import numpy as np

L, E, H, V, LBL = 2048, 1024, 1024, 50257, 3
NCORES = 8
TCONV = 44    # steps per direction for fixed-point convergence (~0.89x/step)
TMAX = 576
CONV_TOL = 4.5e-4

LAST_RESULTS = None  # BassKernelResults of the final run (for test harness)
LAST_NC = None       # compiled Bass program of the final run
LAST_IN_MAPS = None  # per-core input maps of the final run

# device gate-column order within an h-block: i, f, o, g  (torch row blocks
# are i, f, g, o — so col gate q maps to torch block GATE_TORCH[q])
GATE_TORCH = (0, 1, 3, 2)


def _attention(nc, tc, const, work, psum, psum1, T, hist, hfin, lw1_sb, lb1_sb,
               ub_sb, lw2_sb, lb2_sb, out3_d, diag_d, dt, AF, ALU, half_dt):
    """Attention over the T distinct output rows, with the (L-T+1)-fold tail
    multiplicity folded into the softmax weight of row T-1.  The T dimension
    is processed in chunks of <=448 to respect PSUM bank / moving-dim limits."""
    import concourse.mybir as mybir

    CH = 448
    chunks = [(c, min(c + CH, T)) for c in range(0, T, CH)]

    # O^T layout: O_sb[p, t*8+kk] = hs_l[t][kk*128+p] * hs_r[t][kk*128+p]
    O_sb = const.tile([128, T * 8], dt, tag="O")
    nc.vector.tensor_mul(O_sb[:], hist[:, 8:(T + 1) * 8],
                         hist[:, (T + 1) * 8:(2 * T + 1) * 8])
    # tail row (softmax weight ~0.98) recomputed from the fp32 h copies
    nc.vector.tensor_mul(O_sb[:, (T - 1) * 8:T * 8], hfin[:, 8:16],
                         hfin[:, 24:32])
    Ov = O_sb[:].rearrange("p (t kk) -> p kk t", kk=8)
    Oh_sb = const.tile([128, T * 8], half_dt, tag="Oh")
    nc.vector.tensor_copy(Oh_sb[:], O_sb[:])
    Ovh = Oh_sb[:].rearrange("p (t kk) -> p kk t", kk=8)

    # t_matT[h, t] = tanh(sum_hin lin1_w[h, hin] * O^T[hin, t] + b1[h])
    tm_sb = const.tile([128, 8 * T], dt, tag="tm")
    for m in range(8):
        for (c0, c1) in chunks:
            tm_ps = psum.tile([128, c1 - c0], dt, tag="tm_ps", name="tm_ps")
            for kk in range(8):
                nc.tensor.matmul(
                    tm_ps[:],
                    lw1_sb[:, kk, m * 128:(m + 1) * 128],
                    Ovh[:, kk, c0:c1],
                    start=(kk == 0), stop=(kk == 7),
                )
            nc.scalar.activation(tm_sb[:, m * T + c0:m * T + c1], tm_ps[:],
                                 AF.Tanh, bias=lb1_sb[:, m:m + 1])

    # beta row [1, T]
    beta_sb = const.tile([1, T], dt, tag="beta_sb")
    for (c0, c1) in chunks:
        att_ps = psum1.tile([128, 448], dt, tag="att1", name="att1_beta")
        beta_ps = att_ps[0:1, 0:c1 - c0]
        for m in range(8):
            nc.tensor.matmul(beta_ps, ub_sb[:, m:m + 1],
                             tm_sb[:, m * T + c0:m * T + c1],
                             start=(m == 0), stop=(m == 7))
        nc.vector.tensor_copy(beta_sb[:, c0:c1], beta_ps)

    # softmax with tail multiplicity (L - T + 1) on the last row
    bmax = work.tile([1, 1], dt, tag="bmax")
    nc.vector.tensor_reduce(bmax[:], beta_sb[:],
                            axis=mybir.AxisListType.X, op=ALU.max)
    nbmax = work.tile([1, 1], dt, tag="nbmax")
    nc.vector.tensor_scalar_mul(nbmax[:], bmax[:], -1.0)
    ew = work.tile([1, T], dt, tag="ew")
    nc.scalar.activation(ew[:], beta_sb[:], AF.Exp, bias=nbmax[:])
    nc.vector.tensor_scalar_mul(ew[:, T - 1:T], ew[:, T - 1:T],
                                float(L - T + 1))
    denom = work.tile([1, 1], dt, tag="denom")
    nc.vector.tensor_reduce(denom[:], ew[:],
                            axis=mybir.AxisListType.X, op=ALU.add)
    rec = work.tile([1, 1], dt, tag="rec")
    nc.vector.reciprocal(rec[:], denom[:])
    alpha = work.tile([1, T], dt, tag="alpha")
    nc.vector.tensor_scalar_mul(alpha[:], ew[:], rec[:])

    # s[h] = sum_t alpha[t] * O[t, h], chunked with ping-pong accumulator
    ones_sb = const.tile([1, 128], dt, tag="ones")
    nc.vector.memset(ones_sb[:], 1.0)
    s_bufs = [const.tile([128, 8], dt, tag="s0", name="s0"),
              const.tile([128, 8], dt, tag="s1", name="s1")]
    for ci, (c0, c1) in enumerate(chunks):
        att_ps2 = psum1.tile([128, 448], dt, tag="att1", name="att1_ab")
        ab_ps = att_ps2[:, 0:c1 - c0]
        nc.tensor.matmul(ab_ps, ones_sb[:], alpha[:, c0:c1],
                         start=True, stop=True)
        alpha_bc = work.tile([128, c1 - c0], dt, tag="alpha_bc",
                             name="alpha_bc")
        nc.vector.tensor_copy(alpha_bc[:], ab_ps)
        tgt = s_bufs[ci % 2]
        part = (tgt if ci == 0 else
                work.tile([128, 8], dt, tag="s_part", name="s_part"))
        for kk in range(8):
            scratch = work.tile([128, c1 - c0], dt, tag="scratch",
                                name="scratch")
            nc.vector.tensor_mul(scratch[:], Ov[:, kk, c0:c1], alpha_bc[:])
            nc.vector.tensor_reduce(part[:, kk:kk + 1], scratch[:],
                                    axis=mybir.AxisListType.X, op=ALU.add)
        if ci > 0:
            nc.vector.tensor_add(tgt[:], s_bufs[(ci - 1) % 2][:], part[:])
    s_sb = s_bufs[(len(chunks) - 1) % 2]

    # out3 = lin2_w @ s + lin2_b
    att_ps3 = psum1.tile([128, 448], dt, tag="att1", name="att1_o3")
    o3_ps = att_ps3[0:3, 0:1]
    for kk in range(8):
        nc.tensor.matmul(o3_ps, lw2_sb[:, kk, :],
                         s_sb[:, kk:kk + 1],
                         start=(kk == 0), stop=(kk == 7))
    o3_sb = work.tile([3, 1], dt, tag="o3")
    nc.scalar.activation(o3_sb[:], o3_ps, AF.Identity,
                         bias=lb2_sb[:])
    nc.sync.dma_start(out3_d[:], o3_sb[:])

    # convergence diagnostics: fp32 h at slots T-1, T (left) / 2T-1, 2T
    nc.sync.dma_start(diag_d[:], hfin[:])


def _build_program(T, i_star_l, i_star_r):
    """Replicated recurrence with tail-hidden step pipeline.

    PSUM gate layout: col m = hb*4 + q with hb = h-block (0..7), q = gate
    (0=i, 1=f, 2=o, 3=g).  Group A = cols 0..15 (hb 0..3), B = 16..31."""
    import concourse.mybir as mybir
    import concourse.tile as tile
    import concourse.bacc as bacc

    dt = mybir.dt.float32
    bt = mybir.dt.float16
    AF = mybir.ActivationFunctionType
    ALU = mybir.AluOpType

    nc = bacc.Bacc("TRN2", target_bir_lowering=False, debug=False,
                   num_devices=NCORES)

    wl_d = nc.dram_tensor("wl", [1024, 4096], bt, kind="ExternalInput")
    wr_d = nc.dram_tensor("wr", [1024, 4096], bt, kind="ExternalInput")
    z_ds = {}
    for name in ("zcl", "zsl", "zcr", "zsr"):
        z_ds[name] = nc.dram_tensor(name, [128, 32], dt, kind="ExternalInput")
    lw1_d = nc.dram_tensor("lw1", [1024, 1024], bt, kind="ExternalInput")
    lb1_d = nc.dram_tensor("lb1", [128, 8], dt, kind="ExternalInput")
    ub_d = nc.dram_tensor("ub", [128, 8], dt, kind="ExternalInput")
    lw2_d = nc.dram_tensor("lw2", [1024, 3], dt, kind="ExternalInput")
    lb2_d = nc.dram_tensor("lb2", [3, 1], dt, kind="ExternalInput")

    out3_d = nc.dram_tensor("out3", [3, 1], dt, kind="ExternalOutput")
    diag_d = nc.dram_tensor("diag", [128, 32], dt, kind="ExternalOutput")

    with tile.TileContext(nc) as tc:
        with (
            tc.tile_pool(name="const", bufs=1) as const,
            tc.tile_pool(name="work", bufs=3) as work,
            tc.tile_pool(name="psum", bufs=2, space="PSUM") as psum,
            tc.tile_pool(name="psA", bufs=2, space="PSUM") as psA,
            tc.tile_pool(name="psB", bufs=2, space="PSUM") as psB,
            tc.tile_pool(name="psC", bufs=2, space="PSUM") as psC,
            tc.tile_pool(name="psD", bufs=2, space="PSUM") as psD,
            tc.tile_pool(name="psum1", bufs=1, space="PSUM") as psum1,
        ):
            # ---- weights into SBUF, split (kk, half) for early first-MM ----
            wl_sb = const.tile([128, 8, 4096], bt, tag="wl")
            wr_sb = const.tile([128, 8, 4096], bt, tag="wr")
            for (w_sb, w_d) in ((wl_sb, wl_d), (wr_sb, wr_d)):
                wv = w_d[:].rearrange("(kk p) c -> p kk c", p=128)
                for hf, ks in ((1, range(4, 8)), (0, range(4, 8)),
                               (1, range(0, 4)), (0, range(0, 4))):
                    for kk in ks:
                        nc.sync.dma_start(
                            w_sb[:, kk, hf * 2048:(hf + 1) * 2048],
                            wv[:, kk, hf * 2048:(hf + 1) * 2048])
            z_sb = {}
            for name in ("zcl", "zsl", "zcr", "zsr"):
                t_ = const.tile([128, 32], dt, tag=name, name=name)
                nc.sync.dma_start(t_[:], z_ds[name][:])
                z_sb[name] = t_
            lw1_sb = const.tile([128, 8, 1024], bt, tag="lw1")
            lw1v = lw1_d[:].rearrange("(kk p) c -> p kk c", p=128)
            for kk in range(8):
                nc.sync.dma_start(lw1_sb[:, kk, :], lw1v[:, kk, :])
            lb1_sb = const.tile([128, 8], dt, tag="lb1")
            nc.sync.dma_start(lb1_sb[:], lb1_d[:])
            ub_sb = const.tile([128, 8], dt, tag="ub")
            nc.sync.dma_start(ub_sb[:], ub_d[:])
            lw2_sb = const.tile([128, 8, 3], dt, tag="lw2")
            nc.sync.dma_start(lw2_sb[:],
                              lw2_d[:].rearrange("(kk p) c -> p kk c", p=128))
            lb2_sb = const.tile([3, 1], dt, tag="lb2")
            nc.sync.dma_start(lb2_sb[:], lb2_d[:])

            # ---- recurrence state ----
            # hist slot s holds full h after global step s-1 (slot 0 = h0 = 0)
            hist = const.tile([128, (2 * T + 1) * 8], bt, tag="hist")
            nc.vector.memset(hist[:, 0:8], 0.0)
            # fp32 h for slots T-1, T, 2T-1, 2T (diag + attention tail row)
            hfin = const.tile([128, 32], dt, tag="hfin")
            fin_slot = {T - 1: 0, T: 1, 2 * T - 1: 2, 2 * T: 3}
            c_bufs = [const.tile([128, 8], dt, tag="c0", name="c0"),
                      const.tile([128, 8], dt, tag="c1", name="c1")]
            nc.vector.memset(c_bufs[0][:], 0.0)
            # static tail scratch (per half) — avoids per-step pool churn
            tsc = []
            for hf in range(2):
                tsc.append({
                    "gzh": const.tile([128, 16], dt, tag=f"s_gzh{hf}", name=f"s_gzh{hf}"),
                    "gz": const.tile([128, 16], dt, tag=f"s_gz{hf}", name=f"s_gz{hf}"),
                    "ga": const.tile([128, 16], dt, tag=f"s_ga{hf}", name=f"s_ga{hf}"),
                    "tmp": const.tile([128, 4], dt, tag=f"s_tmp{hf}", name=f"s_tmp{hf}"),
                    "t2": const.tile([128, 4], dt, tag=f"s_t2{hf}", name=f"s_t2{hf}"),
                    "tc": const.tile([128, 4], dt, tag=f"s_tc{hf}", name=f"s_tc{hf}"),
                })

            def unit(ap):  # add trailing unit free dim for shape agreement
                return ap.rearrange("p (a b) -> p a b", b=1)

            def step(gs, w_sb, z, zero_h):
                """Global step gs: reads h_bf[gs%2], writes slot gs+1.
                zero_h: h input is exactly 0 -> skip MMs, gates come from z."""
                hb = hist[:, gs * 8:(gs + 1) * 8]
                c_in, c_out = c_bufs[gs % 2], c_bufs[(gs + 1) % 2]
                slot = hist[:, (gs + 1) * 8:(gs + 2) * 8]
                ps_half = None
                if not zero_h:
                    # Four PSUM tiles (per half-group x kk-range) so each
                    # accumulation chain stays contiguous (4 MMs), each tail
                    # half depends only on its own half's MM passes, and the
                    # first 64 MMs of a step depend only on h cols 0..3.
                    ps_half = [
                        (psA.tile([128, 16], dt, tag="psAlo", name="psAlo"),
                         psB.tile([128, 16], dt, tag="psAhi", name="psAhi")),
                        (psC.tile([128, 16], dt, tag="psBlo", name="psBlo"),
                         psD.tile([128, 16], dt, tag="psBhi", name="psBhi")),
                    ]

                def mm_pass(half, ki):
                    ks = range(0, 4) if ki == 0 else range(4, 8)
                    ps = ps_half[half][ki]
                    for m in range(16):
                        for kk in ks:
                            nc.tensor.matmul(
                                ps[:, m:m + 1],
                                w_sb[:, kk,
                                     (half * 16 + m) * 128:
                                     (half * 16 + m + 1) * 128],
                                hist[:, gs * 8 + kk:gs * 8 + kk + 1],
                                start=(kk == ks[0]),
                                stop=(kk == ks[-1]))

                def tail_half(half):
                    eng = nc.vector
                    lo = half * 16
                    hlo = half * 4
                    ts = tsc[half]
                    if zero_h:
                        gzv = z[:, lo:lo + 16].rearrange(
                            "p (hb q) -> p hb q", q=4)
                    else:
                        gzh = ts["gzh"]
                        nc.vector.scalar_tensor_tensor(
                            gzh[:], ps_half[half][1][:], 1.0,
                            z[:, lo:lo + 16],
                            op0=ALU.mult, op1=ALU.add)
                        gz = ts["gz"]
                        nc.vector.tensor_add(gz[:], ps_half[half][0][:],
                                             gzh[:])
                        gzv = gz[:].rearrange("p (hb q) -> p hb q", q=4)
                    ga = ts["ga"]
                    gav = ga[:].rearrange("p (hb q) -> p hb q", q=4)
                    nc.scalar.activation(gav[:, :, 0:3], gzv[:, :, 0:3],
                                         AF.Sigmoid)
                    nc.scalar.activation(gav[:, :, 3:4], gzv[:, :, 3:4],
                                         AF.Tanh)
                    tmp = ts["tmp"]
                    eng.tensor_mul(unit(tmp[:]), gav[:, :, 0:1],
                                   gav[:, :, 3:4])
                    t2 = ts["t2"]
                    eng.tensor_mul(unit(t2[:]),
                                   unit(c_in[:, hlo:hlo + 4]),
                                   gav[:, :, 1:2])
                    eng.tensor_add(c_out[:, hlo:hlo + 4], t2[:], tmp[:])
                    tc_ = ts["tc"]
                    nc.scalar.activation(tc_[:], c_out[:, hlo:hlo + 4],
                                         AF.Tanh)
                    # fp16 h row doubles as next step's MM operand and the
                    # attention history; fp32 kept only for the four slots
                    # feeding the diag check and the attention tail row
                    eng.tensor_mul(unit(slot[:, hlo:hlo + 4]),
                                   gav[:, :, 2:3], unit(tc_[:]))
                    if gs + 1 in fin_slot:
                        fs = fin_slot[gs + 1]
                        eng.tensor_mul(
                            unit(hfin[:, fs * 8 + hlo:fs * 8 + hlo + 4]),
                            gav[:, :, 2:3], unit(tc_[:]))

                # Stream order (B,hi)(B,lo)(A,hi)(A,lo) with tails B then
                # A gives monotone dep-times along the in-order engine
                # queues (the scheduler interleaves the two tails' ops), and
                # the next step's first pass (B,hi) needs only tail-B's h.
                if zero_h:
                    tail_half(1)
                    tail_half(0)
                else:
                    mm_pass(1, 1)
                    mm_pass(0, 1)
                    mm_pass(1, 0)
                    tail_half(1)
                    mm_pass(0, 0)
                    tail_half(0)

            for t in range(T):
                step(t, wl_sb,
                     z_sb["zsl"] if t == i_star_l else z_sb["zcl"],
                     zero_h=(t == 0))
            for t in range(T):
                step(T + t, wr_sb,
                     z_sb["zsr"] if t == i_star_r else z_sb["zcr"],
                     zero_h=False)

            _attention(nc, tc, const, work, psum, psum1, T,
                       hist, hfin, lw1_sb, lb1_sb, ub_sb, lw2_sb, lb2_sb,
                       out3_d, diag_d, dt, AF, ALU, half_dt=bt)

    nc.compile()
    return nc


def prepare(inputs):
    x = np.asarray(inputs["x"])[0].astype(np.int64)
    emb = np.asarray(inputs["emb"], dtype=np.float32)
    start = int(np.asarray(inputs["target_start"])[0])
    end = int(np.asarray(inputs["target_end"])[0])

    w_ih = {"l": np.asarray(inputs["w_ih_l"], np.float32),
            "r": np.asarray(inputs["w_ih_r"], np.float32)}
    w_hh = {"l": np.asarray(inputs["w_hh_l"], np.float32),
            "r": np.asarray(inputs["w_hh_r"], np.float32)}
    b_ih = {"l": np.asarray(inputs["b_ih_l"], np.float32),
            "r": np.asarray(inputs["b_ih_r"], np.float32)}
    b_hh = {"l": np.asarray(inputs["b_hh_l"], np.float32),
            "r": np.asarray(inputs["b_hh_r"], np.float32)}
    lin1_w = np.asarray(inputs["lin1_w"], np.float32)
    lin1_b = np.asarray(inputs["lin1_b"], np.float32)
    u = np.asarray(inputs["u"], np.float32)
    lin2_w = np.asarray(inputs["lin2_w"], np.float32)
    lin2_b = np.asarray(inputs["lin2_b"], np.float32)

    # ---- host prep: target vector and per-step gate-bias contributions ----
    cnt = end - start + 1
    if cnt > 0:
        msum = emb[x[start:end + 1]].sum(axis=0, dtype=np.float32)
    else:
        msum = np.zeros(E, np.float32)
    target = (msum / np.float32(cnt)).astype(np.float32)

    first_l = 0 if start > 0 else end + 1
    first_r = (L - 1) if end < L - 1 else start - 1
    i_star_l = first_l if 0 <= first_l < L else None
    i_star_r = (L - 1 - first_r) if 0 <= first_r < L else None

    def zvec(d, xv):
        return (w_ih[d] @ xv + b_ih[d] + b_hh[d]).astype(np.float32)

    z_const = {d: zvec(d, target) for d in ("l", "r")}
    z_spec = {
        "l": zvec("l", emb[x[first_l]]) if i_star_l is not None else
             np.zeros(4 * H, np.float32),
        "r": zvec("r", emb[x[first_r]]) if i_star_r is not None else
             np.zeros(4 * H, np.float32),
    }

    # perm: device column m = hb*4 + q, q in (i, f, o, g) device order
    perm = np.concatenate([
        GATE_TORCH[m % 4] * H + (m // 4) * 128 + np.arange(128)
        for m in range(32)])
    zfull = lambda zv: np.ascontiguousarray(
        zv[perm].reshape(32, 128).T.astype(np.float32))
    wfull = {d: np.ascontiguousarray(
        w_hh[d][perm, :].T.astype(np.float16)) for d in ("l", "r")}

    lw1_in = np.ascontiguousarray(lin1_w.T.astype(np.float16))
    lb1_in = np.ascontiguousarray(lin1_b.reshape(8, 128).T)
    ub_in = np.ascontiguousarray(u[0].reshape(8, 128).T)
    lw2_in = np.ascontiguousarray(lin2_w.T)
    lb2_in = np.ascontiguousarray(lin2_b.reshape(3, 1))

    in_map = {
        "wl": wfull["l"],
        "wr": wfull["r"],
        "zcl": zfull(z_const["l"]),
        "zsl": zfull(z_spec["l"]),
        "zcr": zfull(z_const["r"]),
        "zsr": zfull(z_spec["r"]),
        "lw1": lw1_in,
        "lb1": lb1_in,
        "ub": ub_in,
        "lw2": lw2_in,
        "lb2": lb2_in,
    }
    in_maps = [in_map for _ in range(NCORES)]

    base = max(i_star_l if i_star_l is not None else 0,
               i_star_r if i_star_r is not None else 0)
    return in_maps, i_star_l, i_star_r, base


def kernel(**inputs):
    global LAST_RESULTS, LAST_NC, LAST_IN_MAPS
    import os
    from concourse import bass_utils

    in_maps, i_star_l, i_star_r, base = prepare(inputs)
    T = min(TMAX, base + TCONV)

    def _run(nc):
        try:
            return bass_utils.run_bass_kernel_spmd(
                nc, in_maps, core_ids=list(range(NCORES)))
        except ModuleNotFoundError:
            # tracing requested but NTFF hook unavailable in this env
            os.environ["BASS_NEVER_TRACE"] = "1"
            return bass_utils.run_bass_kernel_spmd(
                nc, in_maps, core_ids=list(range(NCORES)))

    while True:
        nc = _build_program(T, i_star_l, i_star_r)
        res = _run(nc)
        LAST_RESULTS = res
        diag = res.results[0]["diag"]
        dl = np.abs(diag[:, 8:16] - diag[:, 0:8]).max()
        dr = np.abs(diag[:, 24:32] - diag[:, 16:24]).max()
        if (dl < CONV_TOL and dr < CONV_TOL) or T >= TMAX:
            if not (dl < CONV_TOL and dr < CONV_TOL):
                print(f"kernel: WARNING convergence not reached at T={T} "
                      f"(dl={dl:.2e}, dr={dr:.2e})")
            break
        T = min(TMAX, max(T * 2, base + 2 * TCONV))
        print(f"kernel: convergence check failed (dl={dl:.2e}, dr={dr:.2e}); "
              f"retrying with T={T}")

    LAST_NC = nc
    LAST_IN_MAPS = in_maps
    out = res.results[0]["out3"].reshape(1, 3).astype(np.float32)
    return out


# revision 28
# speedup vs baseline: 1.1424x; 1.1424x over previous
"""Trainium2 Bass kernel for nn_Attention_69483980914985.

Model: bidirectional LSTM (L=2048 steps, H=1024) over a sequence whose input
is a constant vector (mean of target-range embeddings) at every step except
one per direction, followed by softmax attention pooling and a 3-way linear
head.

Because the LSTM input is constant almost everywhere and the gate dynamics
are contractive (rate ~0.89/step), the hidden state converges to a fixed
point in ~56 steps.  The kernel runs T real steps per direction on device,
verifies convergence from device outputs, and folds the (identical) tail
rows into the attention softmax exactly via a multiplicity weight on the
last computed row.

Execution: fully replicated across the 8 cores (a per-step AllGather costs
~26us — far more than the whole replicated 4096-gate step).  Per step the
PE streams all 256 [128x128] fp16 weight tiles (~31-34ns each, instruction-
delivery limited); the activation/cell-update tail is computed in two
half-groups (B = h-blocks 4..7, A = 0..3) whose PSUM accumulation chains
live in four separate single-bank tiles, split by kk range so each chain
stays contiguous (4 MMs) and the next step's first pass depends only on
tail B's h columns:

  step t:  [B:kk4-7][A:kk4-7][B:kk0-3][A:kk0-3]  [B':kk4-7]...
  tails:                     [tail B..........]  [tail A....]

w_ih contributions are step-constant and folded on host; the attention tail
(rows >= T-1, all identical after convergence) enters the softmax with
multiplicity (L - T + 1).
"""

import sys

sys.path.insert(0, "/opt/trn_rl_repo")

import numpy as np

L, E, H, V, LBL = 2048, 1024, 1024, 50257, 3
NCORES = 8
TCONV = 46    # steps per direction for fixed-point convergence (~0.89x/step)
TMAX = 576
CONV_TOL = 3.5e-4

LAST_RESULTS = None  # BassKernelResults of the final run (for test harness)
LAST_NC = None       # compiled Bass program of the final run
LAST_IN_MAPS = None  # per-core input maps of the final run

# device gate-column order within an h-block: i, f, o, g  (torch row blocks
# are i, f, g, o — so col gate q maps to torch block GATE_TORCH[q])
GATE_TORCH = (0, 1, 3, 2)


def _attention(nc, tc, const, work, psum, psum1, T, hist, hfin, lw1_sb, lb1_sb,
               ub_sb, lw2_sb, lb2_sb, out3_d, diag_d, dt, AF, ALU, half_dt):
    """Attention over the T distinct output rows, with the (L-T+1)-fold tail
    multiplicity folded into the softmax weight of row T-1.  The T dimension
    is processed in chunks of <=448 to respect PSUM bank / moving-dim limits."""
    import concourse.mybir as mybir

    CH = 448
    chunks = [(c, min(c + CH, T)) for c in range(0, T, CH)]

    # O^T layout: O_sb[p, t*8+kk] = hs_l[t][kk*128+p] * hs_r[t][kk*128+p]
    O_sb = const.tile([128, T * 8], dt, tag="O")
    nc.vector.tensor_mul(O_sb[:], hist[:, 8:(T + 1) * 8],
                         hist[:, (T + 1) * 8:(2 * T + 1) * 8])
    # tail row (softmax weight ~0.98) recomputed from the fp32 h copies
    nc.vector.tensor_mul(O_sb[:, (T - 1) * 8:T * 8], hfin[:, 8:16],
                         hfin[:, 24:32])
    Ov = O_sb[:].rearrange("p (t kk) -> p kk t", kk=8)
    Oh_sb = const.tile([128, T * 8], half_dt, tag="Oh")
    nc.vector.tensor_copy(Oh_sb[:], O_sb[:])
    Ovh = Oh_sb[:].rearrange("p (t kk) -> p kk t", kk=8)

    # t_matT[h, t] = tanh(sum_hin lin1_w[h, hin] * O^T[hin, t] + b1[h])
    tm_sb = const.tile([128, 8 * T], dt, tag="tm")
    for m in range(8):
        for (c0, c1) in chunks:
            tm_ps = psum.tile([128, c1 - c0], dt, tag="tm_ps", name="tm_ps")
            for kk in range(8):
                nc.tensor.matmul(
                    tm_ps[:],
                    lw1_sb[:, kk, m * 128:(m + 1) * 128],
                    Ovh[:, kk, c0:c1],
                    start=(kk == 0), stop=(kk == 7),
                )
            nc.scalar.activation(tm_sb[:, m * T + c0:m * T + c1], tm_ps[:],
                                 AF.Tanh, bias=lb1_sb[:, m:m + 1])

    # beta row [1, T]
    beta_sb = const.tile([1, T], dt, tag="beta_sb")
    for (c0, c1) in chunks:
        att_ps = psum1.tile([128, 448], dt, tag="att1", name="att1_beta")
        beta_ps = att_ps[0:1, 0:c1 - c0]
        for m in range(8):
            nc.tensor.matmul(beta_ps, ub_sb[:, m:m + 1],
                             tm_sb[:, m * T + c0:m * T + c1],
                             start=(m == 0), stop=(m == 7))
        nc.vector.tensor_copy(beta_sb[:, c0:c1], beta_ps)

    # softmax with tail multiplicity (L - T + 1) on the last row
    bmax = work.tile([1, 1], dt, tag="bmax")
    nc.vector.tensor_reduce(bmax[:], beta_sb[:],
                            axis=mybir.AxisListType.X, op=ALU.max)
    nbmax = work.tile([1, 1], dt, tag="nbmax")
    nc.vector.tensor_scalar_mul(nbmax[:], bmax[:], -1.0)
    ew = work.tile([1, T], dt, tag="ew")
    nc.scalar.activation(ew[:], beta_sb[:], AF.Exp, bias=nbmax[:])
    nc.vector.tensor_scalar_mul(ew[:, T - 1:T], ew[:, T - 1:T],
                                float(L - T + 1))
    denom = work.tile([1, 1], dt, tag="denom")
    nc.vector.tensor_reduce(denom[:], ew[:],
                            axis=mybir.AxisListType.X, op=ALU.add)
    rec = work.tile([1, 1], dt, tag="rec")
    nc.vector.reciprocal(rec[:], denom[:])
    alpha = work.tile([1, T], dt, tag="alpha")
    nc.vector.tensor_scalar_mul(alpha[:], ew[:], rec[:])

    # s[h] = sum_t alpha[t] * O[t, h], chunked with ping-pong accumulator
    ones_sb = const.tile([1, 128], dt, tag="ones")
    nc.vector.memset(ones_sb[:], 1.0)
    s_bufs = [const.tile([128, 8], dt, tag="s0", name="s0"),
              const.tile([128, 8], dt, tag="s1", name="s1")]
    for ci, (c0, c1) in enumerate(chunks):
        att_ps2 = psum1.tile([128, 448], dt, tag="att1", name="att1_ab")
        ab_ps = att_ps2[:, 0:c1 - c0]
        nc.tensor.matmul(ab_ps, ones_sb[:], alpha[:, c0:c1],
                         start=True, stop=True)
        alpha_bc = work.tile([128, c1 - c0], dt, tag="alpha_bc",
                             name="alpha_bc")
        nc.vector.tensor_copy(alpha_bc[:], ab_ps)
        tgt = s_bufs[ci % 2]
        part = (tgt if ci == 0 else
                work.tile([128, 8], dt, tag="s_part", name="s_part"))
        for kk in range(8):
            scratch = work.tile([128, c1 - c0], dt, tag="scratch",
                                name="scratch")
            nc.vector.tensor_mul(scratch[:], Ov[:, kk, c0:c1], alpha_bc[:])
            nc.vector.tensor_reduce(part[:, kk:kk + 1], scratch[:],
                                    axis=mybir.AxisListType.X, op=ALU.add)
        if ci > 0:
            nc.vector.tensor_add(tgt[:], s_bufs[(ci - 1) % 2][:], part[:])
    s_sb = s_bufs[(len(chunks) - 1) % 2]

    # out3 = lin2_w @ s + lin2_b
    att_ps3 = psum1.tile([128, 448], dt, tag="att1", name="att1_o3")
    o3_ps = att_ps3[0:3, 0:1]
    for kk in range(8):
        nc.tensor.matmul(o3_ps, lw2_sb[:, kk, :],
                         s_sb[:, kk:kk + 1],
                         start=(kk == 0), stop=(kk == 7))
    o3_sb = work.tile([3, 1], dt, tag="o3")
    nc.scalar.activation(o3_sb[:], o3_ps, AF.Identity,
                         bias=lb2_sb[:])
    nc.sync.dma_start(out3_d[:], o3_sb[:])

    # convergence diagnostics: fp32 h at slots T-1, T (left) / 2T-1, 2T
    nc.sync.dma_start(diag_d[:], hfin[:])


def _build_program(T, i_star_l, i_star_r):
    """Replicated recurrence with tail-hidden step pipeline.

    PSUM gate layout: col m = hb*4 + q with hb = h-block (0..7), q = gate
    (0=i, 1=f, 2=o, 3=g).  Group A = cols 0..15 (hb 0..3), B = 16..31."""
    import concourse.mybir as mybir
    import concourse.tile as tile
    import concourse.bacc as bacc

    dt = mybir.dt.float32
    bt = mybir.dt.float16
    AF = mybir.ActivationFunctionType
    ALU = mybir.AluOpType

    nc = bacc.Bacc("TRN2", target_bir_lowering=False, debug=False,
                   num_devices=NCORES)

    wl_d = nc.dram_tensor("wl", [1024, 4096], bt, kind="ExternalInput")
    wr_d = nc.dram_tensor("wr", [1024, 4096], bt, kind="ExternalInput")
    z_ds = {}
    for name in ("zcl", "zsl", "zcr", "zsr"):
        z_ds[name] = nc.dram_tensor(name, [128, 32], dt, kind="ExternalInput")
    lw1_d = nc.dram_tensor("lw1", [1024, 1024], bt, kind="ExternalInput")
    lb1_d = nc.dram_tensor("lb1", [128, 8], dt, kind="ExternalInput")
    ub_d = nc.dram_tensor("ub", [128, 8], dt, kind="ExternalInput")
    lw2_d = nc.dram_tensor("lw2", [1024, 3], dt, kind="ExternalInput")
    lb2_d = nc.dram_tensor("lb2", [3, 1], dt, kind="ExternalInput")

    out3_d = nc.dram_tensor("out3", [3, 1], dt, kind="ExternalOutput")
    diag_d = nc.dram_tensor("diag", [128, 32], dt, kind="ExternalOutput")

    with tile.TileContext(nc) as tc:
        with (
            tc.tile_pool(name="const", bufs=1) as const,
            tc.tile_pool(name="work", bufs=3) as work,
            tc.tile_pool(name="psum", bufs=2, space="PSUM") as psum,
            tc.tile_pool(name="psA", bufs=2, space="PSUM") as psA,
            tc.tile_pool(name="psB", bufs=2, space="PSUM") as psB,
            tc.tile_pool(name="psC", bufs=2, space="PSUM") as psC,
            tc.tile_pool(name="psD", bufs=2, space="PSUM") as psD,
            tc.tile_pool(name="psum1", bufs=1, space="PSUM") as psum1,
        ):
            # ---- weights into SBUF, split (kk, half) for early first-MM ----
            wl_sb = const.tile([128, 8, 4096], bt, tag="wl")
            wr_sb = const.tile([128, 8, 4096], bt, tag="wr")
            for (w_sb, w_d) in ((wl_sb, wl_d), (wr_sb, wr_d)):
                wv = w_d[:].rearrange("(kk p) c -> p kk c", p=128)
                for hf, ks in ((1, range(4, 8)), (0, range(4, 8)),
                               (1, range(0, 4)), (0, range(0, 4))):
                    for kk in ks:
                        nc.sync.dma_start(
                            w_sb[:, kk, hf * 2048:(hf + 1) * 2048],
                            wv[:, kk, hf * 2048:(hf + 1) * 2048])
            z_sb = {}
            for name in ("zcl", "zsl", "zcr", "zsr"):
                t_ = const.tile([128, 32], dt, tag=name, name=name)
                nc.sync.dma_start(t_[:], z_ds[name][:])
                z_sb[name] = t_
            lw1_sb = const.tile([128, 8, 1024], bt, tag="lw1")
            lw1v = lw1_d[:].rearrange("(kk p) c -> p kk c", p=128)
            for kk in range(8):
                nc.sync.dma_start(lw1_sb[:, kk, :], lw1v[:, kk, :])
            lb1_sb = const.tile([128, 8], dt, tag="lb1")
            nc.sync.dma_start(lb1_sb[:], lb1_d[:])
            ub_sb = const.tile([128, 8], dt, tag="ub")
            nc.sync.dma_start(ub_sb[:], ub_d[:])
            lw2_sb = const.tile([128, 8, 3], dt, tag="lw2")
            nc.sync.dma_start(lw2_sb[:],
                              lw2_d[:].rearrange("(kk p) c -> p kk c", p=128))
            lb2_sb = const.tile([3, 1], dt, tag="lb2")
            nc.sync.dma_start(lb2_sb[:], lb2_d[:])

            # ---- recurrence state ----
            # hist slot s holds full h after global step s-1 (slot 0 = h0 = 0)
            hist = const.tile([128, (2 * T + 1) * 8], bt, tag="hist")
            nc.vector.memset(hist[:, 0:8], 0.0)
            # fp32 h for slots T-1, T, 2T-1, 2T (diag + attention tail row)
            hfin = const.tile([128, 32], dt, tag="hfin")
            fin_slot = {T - 1: 0, T: 1, 2 * T - 1: 2, 2 * T: 3}
            c_bufs = [const.tile([128, 8], dt, tag="c0", name="c0"),
                      const.tile([128, 8], dt, tag="c1", name="c1")]
            nc.vector.memset(c_bufs[0][:], 0.0)
            # static tail scratch (per half) — avoids per-step pool churn
            tsc = []
            for hf in range(2):
                tsc.append({
                    "gzh": const.tile([128, 16], dt, tag=f"s_gzh{hf}", name=f"s_gzh{hf}"),
                    "gz": const.tile([128, 16], dt, tag=f"s_gz{hf}", name=f"s_gz{hf}"),
                    "ga": const.tile([128, 16], dt, tag=f"s_ga{hf}", name=f"s_ga{hf}"),
                    "tmp": const.tile([128, 4], dt, tag=f"s_tmp{hf}", name=f"s_tmp{hf}"),
                    "t2": const.tile([128, 4], dt, tag=f"s_t2{hf}", name=f"s_t2{hf}"),
                    "tc": const.tile([128, 4], dt, tag=f"s_tc{hf}", name=f"s_tc{hf}"),
                })

            def unit(ap):  # add trailing unit free dim for shape agreement
                return ap.rearrange("p (a b) -> p a b", b=1)

            def step(gs, w_sb, z, zero_h):
                """Global step gs: reads h_bf[gs%2], writes slot gs+1.
                zero_h: h input is exactly 0 -> skip MMs, gates come from z."""
                hb = hist[:, gs * 8:(gs + 1) * 8]
                c_in, c_out = c_bufs[gs % 2], c_bufs[(gs + 1) % 2]
                slot = hist[:, (gs + 1) * 8:(gs + 2) * 8]
                ps_half = None
                if not zero_h:
                    # Four PSUM tiles (per half-group x kk-range) so each
                    # accumulation chain stays contiguous (4 MMs), each tail
                    # half depends only on its own half's MM passes, and the
                    # first 64 MMs of a step depend only on h cols 0..3.
                    ps_half = [
                        (psA.tile([128, 16], dt, tag="psAlo", name="psAlo"),
                         psB.tile([128, 16], dt, tag="psAhi", name="psAhi")),
                        (psC.tile([128, 16], dt, tag="psBlo", name="psBlo"),
                         psD.tile([128, 16], dt, tag="psBhi", name="psBhi")),
                    ]

                def mm_pass(half, ki):
                    ks = range(0, 4) if ki == 0 else range(4, 8)
                    ps = ps_half[half][ki]
                    for m in range(16):
                        for kk in ks:
                            nc.tensor.matmul(
                                ps[:, m:m + 1],
                                w_sb[:, kk,
                                     (half * 16 + m) * 128:
                                     (half * 16 + m + 1) * 128],
                                hist[:, gs * 8 + kk:gs * 8 + kk + 1],
                                start=(kk == ks[0]),
                                stop=(kk == ks[-1]))

                def tail_half(half):
                    eng = nc.vector
                    lo = half * 16
                    hlo = half * 4
                    ts = tsc[half]
                    if zero_h:
                        gzv = z[:, lo:lo + 16].rearrange(
                            "p (hb q) -> p hb q", q=4)
                    else:
                        gzh = ts["gzh"]
                        nc.vector.scalar_tensor_tensor(
                            gzh[:], ps_half[half][1][:], 1.0,
                            z[:, lo:lo + 16],
                            op0=ALU.mult, op1=ALU.add)
                        gz = ts["gz"]
                        nc.vector.tensor_add(gz[:], ps_half[half][0][:],
                                             gzh[:])
                        gzv = gz[:].rearrange("p (hb q) -> p hb q", q=4)
                    ga = ts["ga"]
                    gav = ga[:].rearrange("p (hb q) -> p hb q", q=4)
                    nc.scalar.activation(gav[:, :, 0:3], gzv[:, :, 0:3],
                                         AF.Sigmoid)
                    nc.scalar.activation(gav[:, :, 3:4], gzv[:, :, 3:4],
                                         AF.Tanh)
                    tmp = ts["tmp"]
                    eng.tensor_mul(unit(tmp[:]), gav[:, :, 0:1],
                                   gav[:, :, 3:4])
                    t2 = ts["t2"]
                    eng.tensor_mul(unit(t2[:]),
                                   unit(c_in[:, hlo:hlo + 4]),
                                   gav[:, :, 1:2])
                    eng.tensor_add(c_out[:, hlo:hlo + 4], t2[:], tmp[:])
                    tc_ = ts["tc"]
                    nc.scalar.activation(tc_[:], c_out[:, hlo:hlo + 4],
                                         AF.Tanh)
                    # fp16 h row doubles as next step's MM operand and the
                    # attention history; fp32 kept only for the four slots
                    # feeding the diag check and the attention tail row
                    eng.tensor_mul(unit(slot[:, hlo:hlo + 4]),
                                   gav[:, :, 2:3], unit(tc_[:]))
                    if gs + 1 in fin_slot:
                        fs = fin_slot[gs + 1]
                        eng.tensor_mul(
                            unit(hfin[:, fs * 8 + hlo:fs * 8 + hlo + 4]),
                            gav[:, :, 2:3], unit(tc_[:]))

                # Stream order (B,hi)(B,lo)(A,hi)(A,lo) with tails B then
                # A gives monotone dep-times along the in-order engine
                # queues (the scheduler interleaves the two tails' ops), and
                # the next step's first pass (B,hi) needs only tail-B's h.
                if zero_h:
                    tail_half(1)
                    tail_half(0)
                else:
                    mm_pass(1, 1)
                    mm_pass(0, 1)
                    mm_pass(1, 0)
                    tail_half(1)
                    mm_pass(0, 0)
                    tail_half(0)

            for t in range(T):
                step(t, wl_sb,
                     z_sb["zsl"] if t == i_star_l else z_sb["zcl"],
                     zero_h=(t == 0))
            for t in range(T):
                step(T + t, wr_sb,
                     z_sb["zsr"] if t == i_star_r else z_sb["zcr"],
                     zero_h=False)

            _attention(nc, tc, const, work, psum, psum1, T,
                       hist, hfin, lw1_sb, lb1_sb, ub_sb, lw2_sb, lb2_sb,
                       out3_d, diag_d, dt, AF, ALU, half_dt=bt)

    nc.compile()
    return nc


def prepare(inputs):
    x = np.asarray(inputs["x"])[0].astype(np.int64)
    emb = np.asarray(inputs["emb"], dtype=np.float32)
    start = int(np.asarray(inputs["target_start"])[0])
    end = int(np.asarray(inputs["target_end"])[0])

    w_ih = {"l": np.asarray(inputs["w_ih_l"], np.float32),
            "r": np.asarray(inputs["w_ih_r"], np.float32)}
    w_hh = {"l": np.asarray(inputs["w_hh_l"], np.float32),
            "r": np.asarray(inputs["w_hh_r"], np.float32)}
    b_ih = {"l": np.asarray(inputs["b_ih_l"], np.float32),
            "r": np.asarray(inputs["b_ih_r"], np.float32)}
    b_hh = {"l": np.asarray(inputs["b_hh_l"], np.float32),
            "r": np.asarray(inputs["b_hh_r"], np.float32)}
    lin1_w = np.asarray(inputs["lin1_w"], np.float32)
    lin1_b = np.asarray(inputs["lin1_b"], np.float32)
    u = np.asarray(inputs["u"], np.float32)
    lin2_w = np.asarray(inputs["lin2_w"], np.float32)
    lin2_b = np.asarray(inputs["lin2_b"], np.float32)

    # ---- host prep: target vector and per-step gate-bias contributions ----
    cnt = end - start + 1
    if cnt > 0:
        msum = emb[x[start:end + 1]].sum(axis=0, dtype=np.float32)
    else:
        msum = np.zeros(E, np.float32)
    target = (msum / np.float32(cnt)).astype(np.float32)

    first_l = 0 if start > 0 else end + 1
    first_r = (L - 1) if end < L - 1 else start - 1
    i_star_l = first_l if 0 <= first_l < L else None
    i_star_r = (L - 1 - first_r) if 0 <= first_r < L else None

    def zvec(d, xv):
        return (w_ih[d] @ xv + b_ih[d] + b_hh[d]).astype(np.float32)

    z_const = {d: zvec(d, target) for d in ("l", "r")}
    z_spec = {
        "l": zvec("l", emb[x[first_l]]) if i_star_l is not None else
             np.zeros(4 * H, np.float32),
        "r": zvec("r", emb[x[first_r]]) if i_star_r is not None else
             np.zeros(4 * H, np.float32),
    }

    # perm: device column m = hb*4 + q, q in (i, f, o, g) device order
    perm = np.concatenate([
        GATE_TORCH[m % 4] * H + (m // 4) * 128 + np.arange(128)
        for m in range(32)])
    zfull = lambda zv: np.ascontiguousarray(
        zv[perm].reshape(32, 128).T.astype(np.float32))
    wfull = {d: np.ascontiguousarray(
        w_hh[d][perm, :].T.astype(np.float16)) for d in ("l", "r")}

    lw1_in = np.ascontiguousarray(lin1_w.T.astype(np.float16))
    lb1_in = np.ascontiguousarray(lin1_b.reshape(8, 128).T)
    ub_in = np.ascontiguousarray(u[0].reshape(8, 128).T)
    lw2_in = np.ascontiguousarray(lin2_w.T)
    lb2_in = np.ascontiguousarray(lin2_b.reshape(3, 1))

    in_map = {
        "wl": wfull["l"],
        "wr": wfull["r"],
        "zcl": zfull(z_const["l"]),
        "zsl": zfull(z_spec["l"]),
        "zcr": zfull(z_const["r"]),
        "zsr": zfull(z_spec["r"]),
        "lw1": lw1_in,
        "lb1": lb1_in,
        "ub": ub_in,
        "lw2": lw2_in,
        "lb2": lb2_in,
    }
    in_maps = [in_map for _ in range(NCORES)]

    base = max(i_star_l if i_star_l is not None else 0,
               i_star_r if i_star_r is not None else 0)
    return in_maps, i_star_l, i_star_r, base


def kernel(**inputs):
    global LAST_RESULTS, LAST_NC, LAST_IN_MAPS
    import os
    from concourse import bass_utils

    in_maps, i_star_l, i_star_r, base = prepare(inputs)
    T = min(TMAX, base + TCONV)

    def _run(nc):
        try:
            return bass_utils.run_bass_kernel_spmd(
                nc, in_maps, core_ids=list(range(NCORES)))
        except ModuleNotFoundError:
            # tracing requested but NTFF hook unavailable in this env
            os.environ["BASS_NEVER_TRACE"] = "1"
            return bass_utils.run_bass_kernel_spmd(
                nc, in_maps, core_ids=list(range(NCORES)))

    while True:
        nc = _build_program(T, i_star_l, i_star_r)
        res = _run(nc)
        LAST_RESULTS = res
        diag = res.results[0]["diag"]
        dl = np.abs(diag[:, 8:16] - diag[:, 0:8]).max()
        dr = np.abs(diag[:, 24:32] - diag[:, 16:24]).max()
        if (dl < CONV_TOL and dr < CONV_TOL) or T >= TMAX:
            if not (dl < CONV_TOL and dr < CONV_TOL):
                print(f"kernel: WARNING convergence not reached at T={T} "
                      f"(dl={dl:.2e}, dr={dr:.2e})")
            break
        T = min(TMAX, max(T * 2, base + 2 * TCONV))
        print(f"kernel: convergence check failed (dl={dl:.2e}, dr={dr:.2e}); "
              f"retrying with T={T}")

    LAST_NC = nc
    LAST_IN_MAPS = in_maps
    out = res.results[0]["out3"].reshape(1, 3).astype(np.float32)
    return out
